# revision 13
# baseline (speedup 1.0000x reference)
"""Trainium2 Bass kernel for nn_DNN_Model_33852932227151.

Per-sample pipeline (see reference):
  theta1 = MLP(sample1)            303 -> 1024 -> 1024 -> 512 -> 264
  F1, F2 normalized precoders      (cols 200:264)
  theta  = unit-modulus phases     (cols 0:200 as complex [100])
  CCC_bc = Re(theta^H T_bc theta) / 1e-15 ; scale = rsqrt(max(max_c CCC, 1))
  out    = [Re(theta*scale), Im(theta*scale), Re F1, Im F1, Re F2, Im F2]

Sharding: pure data parallel over batch: 2048 = 8 cores x 256 samples.

Design notes (v2):
  - Everything on the theta->quad path is fp32. The quadratic form is scaled
    by 1e15 and clamped at 1, so a bf16 contraction (abs err ~0.3) flips the
    clamp for samples whose true max-quad lands near 0 -> catastrophically
    wrong scale. fp32 keeps the dead-zone probability negligible.
  - True per-core HBM floor is ~91 MB (82 MB of T + 8.6 MB weights) ~ 255us.
    The schedule keeps DMA continuous: weight+sample loads on the scalar
    HWDGE ring, T chunks on the sync HWDGE ring, group-0 MLP first (N=128)
    so streaming starts ~55us in.
  - Per (group, h) unit: 8 fp32 T chunks [128, 25, 100]. DVE does the
    chunk*outer multiplies, ACT does the accumulate-reduce, and the NEXT
    unit's outer-product build is spread over GpSimd/ACT/DVE so no engine
    exceeds the 28.6us/unit DMA arrival rate.
  - Outer build per unit: np1 = -a_n*b_m, t_ba = b_n*a_m, o1 = s_n*s_m
    (s=a+b), then o2 = t_ba+np1, o1 += np1, o1 -= t_ba. ACT builds rows via
    activation(Copy, scale=per-partition AP); GP/DVE build via broadcast TT.
"""

import os
import threading

import numpy as np

import concourse.bass as bass
from concourse import bacc
import concourse.mybir as mybir
import concourse.tile as tile
from concourse.bass_utils import run_bass_kernel_spmd

F32 = mybir.dt.float32

# ---- problem constants (hardcoded per harness contract) ----
B = 2048
N_CORES = 8
B_LOC = B // N_CORES          # 256 samples per core
DIN = 303
H1, H2, H3 = 1024, 1024, 512
DOUT = 264
NRIS = 100
C = 4
MN = 16
INV_THRESH = 1.0e15           # 1 / THRESH_W

N_GROUPS = B_LOC // 128       # 2 sample groups of 128 per core

ROWS = int(os.environ.get("KERNEL_ROWS", "25"))
N_H = NRIS // ROWS
TCH_BUFS = int(os.environ.get("KERNEL_TCH_BUFS", "5"))
# engine for each build op: muls (tmp=a.b, o2=b.a, o1=s.s) from {gp, dve, act},
# adds (o1-=tmp, o1-=o2, o2-=tmp) from {gp, dve}
BUILD_MULS = os.environ.get("KERNEL_BUILD_MULS", "act,act,gp").split(",")
BUILD_ADDS = os.environ.get("KERNEL_BUILD_ADDS", "gp,dve,gp").split(",")
# contraction reduce: "ttr" = fused DVE tensor_tensor_reduce,
# "stt" = fused DVE scalar_tensor_tensor(accum_out),
# "act" = DVE mul + scalar-engine accumulate
ACCUM_PATH = os.environ.get("KERNEL_ACCUM", "stt")
# debug bisect: "full" | "noquad" (skip T contraction; scale=1)
STAGE = os.environ.get("KERNEL_STAGE", "full")


def build_nc():
    nc = bacc.Bacc(trn_type="TRN2", debug=False)

    s1 = nc.declare_dram_parameter("sample1", [B_LOC, DIN], F32, isOutput=False)
    t_re = nc.declare_dram_parameter("T_real", [B_LOC, C, NRIS, NRIS], F32, isOutput=False)
    t_im = nc.declare_dram_parameter("T_imag", [B_LOC, C, NRIS, NRIS], F32, isOutput=False)
    w1 = nc.declare_dram_parameter("W1", [DIN, H1], F32, isOutput=False)
    b1 = nc.declare_dram_parameter("b1", [H1], F32, isOutput=False)
    w2 = nc.declare_dram_parameter("W2", [H1, H2], F32, isOutput=False)
    b2 = nc.declare_dram_parameter("b2", [H2], F32, isOutput=False)
    w3 = nc.declare_dram_parameter("W3", [H2, H3], F32, isOutput=False)
    b3 = nc.declare_dram_parameter("b3", [H3], F32, isOutput=False)
    w4 = nc.declare_dram_parameter("W4", [H3, DOUT], F32, isOutput=False)
    b4 = nc.declare_dram_parameter("b4", [DOUT], F32, isOutput=False)
    out = nc.declare_dram_parameter("out", [B_LOC, DOUT], F32, isOutput=True)

    ident_dram = nc.inline_tensor(np.eye(128, dtype=np.float32), name="ident128")

    with tile.TileContext(nc) as tc:
        _emit(tc, s1, t_re, t_im, (w1, b1), (w2, b2), (w3, b3), (w4, b4),
              out, ident_dram)
    nc.compile()
    return nc


def _emit(tc, s1, t_re, t_im, l1, l2, l3, l4, out, ident_dram):
    nc = tc.nc
    w1, b1 = l1
    w2, b2 = l2
    w3, b3 = l3
    w4, b4 = l4

    with (
        tc.tile_pool(name="consts", bufs=1) as consts,
        tc.tile_pool(name="acts", bufs=1) as acts,
        tc.tile_pool(name="theta", bufs=1) as theta_pool,
        tc.tile_pool(name="tsc", bufs=2) as tsc_pool,
        tc.tile_pool(name="tch", bufs=TCH_BUFS) as tch_pool,
        tc.tile_pool(name="ob2", bufs=2) as ob2,        # o1, o2 (double-buffered)
        tc.tile_pool(name="ob1", bufs=1) as ob1,        # np1, t_ba, dump
        tc.tile_pool(name="wpool", bufs=1) as wpool,
        tc.tile_pool(name="psmm", bufs=4, space="PSUM") as psmm,
        tc.tile_pool(name="pstr", bufs=2, space="PSUM") as pstr,
    ):
        # ---------------- front-loaded DMAs (sync HWDGE ring) ----------------
        # Everything on the sync ring, ordered so each MLP layer's weights
        # land just before they're needed; T chunks queue FIFO behind them.
        # The scalar ring carries only the (small) output DMAs so the ACT
        # sequencer never stalls compute behind DMA descriptor generation.
        ident = consts.tile([128, 128], F32)
        nc.sync.dma_start(out=ident, in_=ident_dram[:, :])

        s1_nats = []
        for bt in range(2):
            s1_nat = tsc_pool.tile([128, DIN], F32, tag="s1nat")
            nc.sync.dma_start(out=s1_nat, in_=s1[bt * 128:(bt + 1) * 128, :])
            s1_nats.append(s1_nat)

        w1s = wpool.tile([128, 3, H1], F32)
        nc.vector.memset(w1s[:, 2, :], 0.0)
        nc.sync.dma_start(out=w1s[:, 0, :], in_=w1[0:128, :])
        nc.sync.dma_start(out=w1s[:, 1, :], in_=w1[128:256, :])
        nc.sync.dma_start(out=w1s[0:47, 2, :], in_=w1[256:303, :])
        b1s = wpool.tile([128, 8], F32)
        nc.sync.dma_start(out=b1s, in_=b1[:].rearrange("(o p) -> p o", p=128))
        b2s = wpool.tile([128, 8], F32)
        nc.sync.dma_start(out=b2s, in_=b2[:].rearrange("(o p) -> p o", p=128))
        b3s = wpool.tile([128, 4], F32)
        nc.sync.dma_start(out=b3s, in_=b3[:].rearrange("(o p) -> p o", p=128))
        b4s = wpool.tile([128, 3], F32)
        nc.sync.dma_start(out=b4s[0:100, 0:1], in_=b4[0:100, None])
        nc.sync.dma_start(out=b4s[0:100, 1:2], in_=b4[100:200, None])
        nc.sync.dma_start(out=b4s[0:64, 2:3], in_=b4[200:264, None])
        w2s = wpool.tile([128, 8, H2], F32)
        nc.sync.dma_start(out=w2s, in_=w2[:, :].rearrange("(o p) m -> p o m", p=128))
        w3s = wpool.tile([128, 8, H3], F32)
        nc.sync.dma_start(out=w3s, in_=w3[:, :].rearrange("(o p) m -> p o m", p=128))
        w4s = wpool.tile([128, 4, DOUT], F32)
        nc.sync.dma_start(out=w4s, in_=w4[:, :].rearrange("(o p) m -> p o m", p=128))

        # ---------------- persistent tiles ----------------
        x0 = wpool.tile([128, 3, B_LOC], F32)
        nc.vector.memset(x0[:, 2, :], 0.0)
        h1t = wpool.tile([128, 8, B_LOC], F32)
        h2t = wpool.tile([128, 8, B_LOC], F32)
        h3t = wpool.tile([128, 4, B_LOC], F32)
        thp = acts.tile([128, 3, B_LOC], F32)  # [:,0]=re, [:,1]=im, [:,2]=F(64)
        nc.vector.memset(thp[64:128, 2, :], 0.0)
        a_fm = theta_pool.tile([128, B_LOC], F32)
        b_fm = theta_pool.tile([128, B_LOC], F32)
        nc.vector.memset(a_fm, 0.0)
        nc.vector.memset(b_fm, 0.0)

        def dense(gs, in_tile, n_k, ws, n_m, bias_s, relu, out_tile, m_widths=None):
            # out[feat, gs] = act(W.T @ in[:, :, gs] + b); K = n_k*128 partitions
            for mo in range(n_m):
                if m_widths is None:
                    mw, m_lo = 128, mo * 128
                else:
                    m_lo, mw = m_widths[mo]
                ps = psmm.tile([128, 128], F32, tag="mm")
                for k in range(n_k):
                    nc.tensor.matmul(
                        ps[0:mw, :],
                        lhsT=ws[:, k, m_lo:m_lo + mw],
                        rhs=in_tile[:, k, gs],
                        start=(k == 0),
                        stop=(k == n_k - 1),
                    )
                if relu:
                    nc.scalar.activation(
                        out=out_tile[0:mw, mo, gs], in_=ps[0:mw, :],
                        func=mybir.ActivationFunctionType.Relu,
                        bias=bias_s[0:mw, mo:mo + 1], scale=1.0)
                else:
                    nc.vector.tensor_scalar(
                        out=out_tile[0:mw, mo, gs], in0=ps[0:mw, :],
                        scalar1=bias_s[0:mw, mo:mo + 1], scalar2=None,
                        op0=mybir.AluOpType.add)

        # per-group state
        gstate = [dict() for _ in range(N_GROUPS)]

        def emit_mlp_group(g):
            gs = slice(g * 128, (g + 1) * 128)
            # input transpose for this group's samples
            for ft in range(3):
                w = min(128, DIN - ft * 128)
                ps = pstr.tile([128, 128], F32, tag="tr")
                nc.tensor.transpose(ps[0:w, :],
                                    s1_nats[g][:, ft * 128:ft * 128 + w],
                                    ident)
                nc.scalar.copy(out=x0[0:w, ft, gs], in_=ps[0:w, :])
            dense(gs, x0, 3, w1s, 8, b1s, True, h1t)
            dense(gs, h1t, 8, w2s, 8, b2s, True, h2t)
            dense(gs, h2t, 8, w3s, 4, b3s, True, h3t)
            dense(gs, h3t, 4, w4s, 3, b4s, False, thp,
                  m_widths=[(0, 100), (100, 100), (200, 64)])

        def emit_theta_group(g):
            """unit-modulus theta + sample-major transposes + F normalization"""
            gs = slice(g * 128, (g + 1) * 128)
            st = gstate[g]
            p_re = thp[0:100, 0, gs]
            p_im = thp[0:100, 1, gs]
            sq = tsc_pool.tile([100, 128], F32, tag="sq")
            sq2 = tsc_pool.tile([100, 128], F32, tag="sq2")
            nc.vector.tensor_mul(sq, p_re, p_re)
            nc.vector.tensor_mul(sq2, p_im, p_im)
            nc.vector.tensor_add(sq, sq, sq2)
            nc.scalar.sqrt(sq, sq)
            nc.vector.reciprocal(sq, sq)               # 1/|theta|
            nc.vector.tensor_mul(a_fm[0:100, gs], p_re, sq)
            nc.vector.tensor_mul(b_fm[0:100, gs], p_im, sq)

            def to_sample_major(src_fm, np_, tag):
                ps = pstr.tile([128, 128], F32, tag="tr")
                nc.tensor.transpose(ps, src_fm, ident)
                dst = theta_pool.tile([128, np_], F32, tag=tag)
                nc.scalar.copy(out=dst, in_=ps[:, 0:np_])
                return dst

            a_pack = to_sample_major(a_fm[:, gs], 100, f"apack{g}")
            b_pack = to_sample_major(b_fm[:, gs], 100, f"bpack{g}")
            f_pack = to_sample_major(thp[:, 2, gs], 64, f"fpack{g}")
            st["a"], st["b"] = a_pack, b_pack

            s_pack = theta_pool.tile([128, NRIS], F32, tag=f"spack{g}")
            nc.vector.tensor_add(s_pack, a_pack, b_pack)
            st["s"] = s_pack

            # ---- F1/F2 precoder normalization ----
            fsq = tsc_pool.tile([128, 2, 32], F32, tag="fsq")
            f_v = f_pack[:].rearrange("p (g2 i) -> p g2 i", g2=2)
            nc.vector.tensor_mul(fsq, f_v, f_v)
            fnorm = tsc_pool.tile([128, 2], F32, tag="fnorm")
            nc.vector.reduce_sum(fnorm, fsq, axis=mybir.AxisListType.X)
            # scale = sqrt(2/norm) = 1/sqrt(norm*0.5)
            nc.scalar.activation(out=fnorm, in_=fnorm,
                                 func=mybir.ActivationFunctionType.Sqrt, scale=0.5)
            nc.vector.reciprocal(fnorm, fnorm)
            fhat = theta_pool.tile([128, 2, 32], F32, tag=f"fhat{g}")
            nc.vector.tensor_mul(fhat, f_v,
                                 fnorm[:, :, None].to_broadcast((128, 2, 32)))
            nc.scalar.dma_start(out=out[gs, 200:264],
                                in_=fhat[:].rearrange("p g2 i -> p (g2 i)"))

            parts = theta_pool.tile([128, C, 2 * N_H], F32, tag=f"parts{g}")
            st["parts"] = parts

        # ---------------- outer-product build (one unit = (g, h)) -------------
        TT = mybir.AluOpType

        def build_unit_ops(g, h):
            """Return ([6 closures], (o1, o2)) building this unit's outers.

            o1 = aa^T + bb^T = ss^T - ab^T - ba^T   (s = a + b)
            o2 = ba^T - ab^T
            via: tmp = ab^T; o2 = ba^T; o1 = ss^T; o1 -= tmp; o1 -= o2;
                 o2 -= tmp   (the o1 -= o2 read precedes the o2 update)
            """
            st = gstate[g]
            a, b, s = st["a"], st["b"], st["s"]
            hs = slice(h * ROWS, (h + 1) * ROWS)
            sh3 = (128, ROWS, NRIS)
            o1 = ob2.tile([128, ROWS, NRIS], F32, tag="o1")
            o2 = ob2.tile([128, ROWS, NRIS], F32, tag="o2")
            tmp = ob1.tile([128, ROWS, NRIS], F32, tag="tmp")

            def mul_op(eng, dst, row_scale_full, row_vec):
                # dst[s, n, m] = row_scale_full[s, h*ROWS+n] * row_vec[s, m]
                if eng == "act":
                    def f():
                        for n in range(ROWS):
                            nc.scalar.activation(
                                out=dst[:, n, :], in_=row_vec,
                                func=mybir.ActivationFunctionType.Copy,
                                bias=0.0,
                                scale=row_scale_full[:, h * ROWS + n:h * ROWS + n + 1])
                else:
                    e = nc.gpsimd if eng == "gp" else nc.vector
                    def f():
                        e.tensor_mul(dst,
                                     row_scale_full[:, hs, None].to_broadcast(sh3),
                                     row_vec[:, None, :].to_broadcast(sh3))
                return f

            def add_op(eng, dst, x, y, op):
                e = nc.gpsimd if eng == "gp" else nc.vector
                def f():
                    e.tensor_tensor(dst, x, y, op)
                return f

            ops = [
                mul_op(BUILD_MULS[0], tmp, a, b),         # tmp = a_n b_m
                mul_op(BUILD_MULS[1], o2, b, a),          # o2 = b_n a_m
                mul_op(BUILD_MULS[2], o1, s, s),          # o1 = s_n s_m
                add_op(BUILD_ADDS[0], o1, o1, tmp, TT.subtract),
                add_op(BUILD_ADDS[1], o1, o1, o2, TT.subtract),
                add_op(BUILD_ADDS[2], o2, o2, tmp, TT.subtract),
            ]
            return ops, (o1, o2)

        # ---------------- group finale ----------------
        def emit_finale(g):
            gs = slice(g * 128, (g + 1) * 128)
            st = gstate[g]
            th = theta_pool.tile([128, 2, NRIS], F32, tag=f"th{g}")
            mx = tsc_pool.tile([128, 1], F32, tag="mx")
            if STAGE == "noquad":
                nc.vector.memset(mx, 1.0)
            else:
                ccc = tsc_pool.tile([128, C], F32, tag="ccc")
                nc.vector.reduce_sum(ccc, st["parts"], axis=mybir.AxisListType.X)
                nc.vector.reduce_max(mx, ccc, axis=mybir.AxisListType.X)
                # scale = rsqrt(max(mx*1e15, 1)) = rsqrt(1e15 * max(mx, 1e-15))
                nc.vector.tensor_scalar(out=mx, in0=mx, scalar1=1.0 / INV_THRESH,
                                        scalar2=None, op0=mybir.AluOpType.max)
                nc.scalar.activation(out=mx, in_=mx,
                                     func=mybir.ActivationFunctionType.Sqrt,
                                     scale=INV_THRESH)
                nc.vector.reciprocal(mx, mx)
            nc.vector.tensor_scalar_mul(th[:, 0, :], st["a"], mx)
            nc.vector.tensor_scalar_mul(th[:, 1, :], st["b"], mx)
            nc.scalar.dma_start(out=out[gs, 0:200],
                                in_=th[:].rearrange("p r n -> p (r n)"))

        # ---------------- emission schedule ----------------
        emit_mlp_group(0)
        emit_theta_group(0)

        units = [(g, h) for g in range(N_GROUPS) for h in range(N_H)]

        # build unit 0 up front
        ops0, outers = build_unit_ops(*units[0])
        for f in ops0:
            f()

        for ui, (g, h) in enumerate(units):
            o1, o2 = outers
            # defer next unit's build into this unit's chunk slots
            pending = []
            if ui + 1 < len(units):
                ng, nh = units[ui + 1]
                if (ng, nh) == (1, 0):
                    # group 1 theta must exist before its build ops
                    emit_mlp_group(1)
                    emit_theta_group(1)
                pending, outers = build_unit_ops(ng, nh)
            st = gstate[g]
            parts = st["parts"]
            slot = 0
            for c in range(C):
                for ri, (t_dram, o_t) in enumerate(((t_re, o1), (t_im, o2))):
                    chunk = tch_pool.tile([128, ROWS, NRIS], F32, tag="tchunk")
                    nc.sync.dma_start(
                        out=chunk,
                        in_=t_dram[g * 128:(g + 1) * 128, c,
                                   h * ROWS:(h + 1) * ROWS, :])
                    if STAGE == "noquad":
                        continue
                    acc = parts[:, c, ri * N_H + h:ri * N_H + h + 1]
                    if ACCUM_PATH == "ttr":
                        nc.vector.tensor_tensor_reduce(
                            out=chunk, in0=chunk, in1=o_t,
                            scale=1.0, scalar=0.0,
                            op0=mybir.AluOpType.mult, op1=mybir.AluOpType.add,
                            accum_out=acc)
                    elif ACCUM_PATH == "stt":
                        # out = (chunk add 0) mult o_t ; acc = sum(out)
                        nc.vector.scalar_tensor_tensor(
                            out=chunk, in0=chunk, scalar=0.0, in1=o_t,
                            op0=mybir.AluOpType.add, op1=mybir.AluOpType.mult,
                            accum_out=acc)
                    else:
                        nc.vector.tensor_mul(chunk, chunk, o_t)
                        nc.scalar.activation(
                            out=chunk, in_=chunk,
                            func=mybir.ActivationFunctionType.Copy,
                            bias=0.0, scale=1.0, accum_out=acc)
                    if pending:
                        pending.pop(0)()
                    slot += 1
            for f in pending:
                f()
            if h == N_H - 1:
                emit_finale(g)


_NC_LOCK = threading.Lock()
_NC = None


def _get_nc():
    global _NC
    with _NC_LOCK:
        if _NC is None:
            _NC = build_nc()
    return _NC


def _shard_inputs(inputs):
    in_maps = []
    for i in range(N_CORES):
        bs = slice(i * B_LOC, (i + 1) * B_LOC)
        in_maps.append({
            "sample1": np.ascontiguousarray(inputs["sample1"][bs]),
            "T_real": np.ascontiguousarray(inputs["T_real"][bs]),
            "T_imag": np.ascontiguousarray(inputs["T_imag"][bs]),
            "W1": np.asarray(inputs["W1"]), "b1": np.asarray(inputs["b1"]),
            "W2": np.asarray(inputs["W2"]), "b2": np.asarray(inputs["b2"]),
            "W3": np.asarray(inputs["W3"]), "b3": np.asarray(inputs["b3"]),
            "W4": np.asarray(inputs["W4"]), "b4": np.asarray(inputs["b4"]),
        })
    return in_maps


def run_on_hw(inputs, trace=False, **kwargs):
    nc = _get_nc()
    res = run_bass_kernel_spmd(nc, _shard_inputs(inputs),
                               list(range(N_CORES)), trace=trace, **kwargs)
    full = np.concatenate([res.results[i]["out"] for i in range(N_CORES)], axis=0)
    return full, res


def kernel(**inputs) -> np.ndarray:
    full, _ = run_on_hw(inputs, trace=False)
    return full.astype(np.float32)
